# revision 25
# baseline (speedup 1.0000x reference)
"""Deformable Conv2D Trainium2 kernel (8-core data-parallel over batch).

Per core (one image, H=W=128, C=64, F=128, 3x3 deformable conv):
  Phase A (all 16 bands up front, so gathers never wait):
    1. offset conv (PE, fp16, K-packed dual-tap matmuls)
    2. offsets transposed to pixel-major (PE identity matmuls)
    3. bilinear weights + gather indices (DVE, pixel-major)
    4. index fold to the DMA-gather wrapped layout (PE transposes + DVE)
       -> idxb_all [128, 16*576] i16, w36_all [128, 16*288] fp16
  Phase B (per band, Q7-gather-rate limited):
    5. dma_gather of 512B fp16 quad-corner tokens from the row-paired
       DRAM image (xpair[y*128+x] = img[y,x] ++ img[y+1,x]) - one
       descriptor per (pixel, tap)
    6. bilinear combine: corner weights broadcast across channels
       (0-stride ACT copy), one wide DVE multiply + 2 strided adds
    7. sampled transposed to channel-major via PE identity matmuls into a
       halo'd per-band buffer
    8. main conv: 45 accumulating PE matmuls per 512-px chunk (fp16)
    9. output stored channel-major [F, H*W]; host transposes to NHWC

Self-contained: hardcodes shapes for the nn_DeformableConv2D problem.
"""
import os
import numpy as np

import concourse.bass as bass
import concourse.bacc as bacc
import concourse.tile as tile
from concourse import mybir
from concourse.bass_utils import run_bass_kernel_spmd

F32, F16, I16 = mybir.dt.float32, mybir.dt.float16, mybir.dt.int16
ALU = mybir.AluOpType
ACTF = mybir.ActivationFunctionType

H = WD = 128
C = 64
F = 128
T = 9            # deformable taps
NCORES = 8
ROWS_PER_BAND = 8
BANDS = H // ROWS_PER_BAND          # 16
UNITS = 4                            # 2-row units per band
PXROW = WD                            # 128 px per image row
PAD = 130                            # padded row length for shifted reads
NPOS = 16512                         # xpair positions (129 rows x 128)
KB = 5                               # K blocks of main conv (576 -> 640)
SLOT = PAD                           # 130 cols per row slot in scm
SCMW = KB * 10 * SLOT                # 6500 cols per band buffer
IDX_PER_UNIT = 2 * T * PXROW         # 2304 tokens per 2-row unit

_CACHE = {}


STAGE = int(os.environ.get("KSTAGE", "4"))
GCHUNK = int(os.environ.get("KGCHUNK", "1024"))
SINGLE_PACKET = os.environ.get("KSP", "1") == "1"
NQUEUES = int(os.environ.get("KNQ", "4"))


def build_program():
    if "nc" in _CACHE:
        return _CACHE["nc"]
    nc = bacc.Bacc("TRN2", target_bir_lowering=False, debug=False,
                   num_swdge_queues=NQUEUES)

    # ---- DRAM I/O ----
    xpair = nc.dram_tensor("xpair", [NPOS * 128], F16, kind="ExternalInput").ap()
    xdup = nc.dram_tensor("xdup", [128, PAD * PAD], F16, kind="ExternalInput").ap()
    woffd_in = nc.dram_tensor("woffd", [128, 3 * 18], F16, kind="ExternalInput").ap()
    woffs_in = nc.dram_tensor("woffs", [64, 3 * 18], F16, kind="ExternalInput").ap()
    wm_in = nc.dram_tensor("wm", [128, 45 * 128], F16, kind="ExternalInput").ap()
    cx_in = nc.dram_tensor("cx", [128, H * T], F16, kind="ExternalInput").ap()
    cy_in = nc.dram_tensor("cy", [128, H * T], F16, kind="ExternalInput").ap()
    i32_in = nc.dram_tensor("i128f", [128, 128], F32, kind="ExternalInput").ap()
    i16_in = nc.dram_tensor("i128h", [128, 128], F16, kind="ExternalInput").ap()
    b_in = nc.dram_tensor("b_main", [128, 1], F32, kind="ExternalInput").ap()
    boff_in = nc.dram_tensor("b_off", [18, 1], F32, kind="ExternalInput").ap()
    out_dram = nc.dram_tensor("out", [F, H * WD], F32, kind="ExternalOutput").ap()
    dbg = nc.dram_tensor("dbg", [128, 4608], F32, kind="ExternalOutput").ap()

    with tile.TileContext(nc) as tc:
        _emit(nc, tc, xpair, xdup, woffd_in, woffs_in, wm_in, cx_in, cy_in,
              i32_in, i16_in, b_in, boff_in, out_dram, dbg)

    nc.compile()
    _CACHE["nc"] = nc
    return nc


def _emit(nc, tc, xpair, xdup_in, woffd_in, woffs_in, wm_in, cx_in, cy_in,
          i32_in, i16_in, b_in, boff_in, out_dram, dbg):
    from contextlib import ExitStack
    with ExitStack() as ctx:
        ec = ctx.enter_context
        st = ec(tc.tile_pool(name="static", bufs=1))
        p_offs = ec(tc.tile_pool(name="offs", bufs=2))
        p_offb = ec(tc.tile_pool(name="offb", bufs=1))
        p_math = ec(tc.tile_pool(name="math", bufs=2))
        p_w = ec(tc.tile_pool(name="wts", bufs=2))
        p_fold = ec(tc.tile_pool(name="fold", bufs=1))
        p_gt = ec(tc.tile_pool(name="gt", bufs=3))
        p_w4 = ec(tc.tile_pool(name="w4", bufs=2))
        p_cmb = ec(tc.tile_pool(name="cmb", bufs=2))
        p_spx = ec(tc.tile_pool(name="spx", bufs=2))
        p_out = ec(tc.tile_pool(name="outp", bufs=2))
        p_dbg = ec(tc.tile_pool(name="dbgp", bufs=1)) if STAGE <= 3 else None
        psA = ec(tc.tile_pool(name="psA", bufs=2, space="PSUM"))
        psB = ec(tc.tile_pool(name="psB", bufs=2, space="PSUM"))
        psS = ec(tc.tile_pool(name="psS", bufs=2, space="PSUM"))
        psC = ec(tc.tile_pool(name="psC", bufs=2, space="PSUM"))

        # ---- static loads (HWDGE; keep Q7 free for gathers) ----
        xdup = st.tile([128, PAD * PAD], F16)
        HEADC = 12 * PAD
        nc.sync.dma_start(xdup[:, 0:HEADC],
                          bass.AP(xdup_in.tensor, 0, [[PAD * PAD, 128], [1, HEADC]]))
        nc.sync.dma_start(xdup[:, HEADC:],
                          bass.AP(xdup_in.tensor, HEADC,
                                  [[PAD * PAD, 128], [1, PAD * PAD - HEADC]]))
        woffd = st.tile([128, 54], F16)
        nc.sync.dma_start(woffd[:], woffd_in)
        woffs = st.tile([64, 54], F16)
        nc.sync.dma_start(woffs[:], woffs_in)
        wm = st.tile([128, 45 * 128], F16)
        nc.sync.dma_start(wm[:], wm_in)
        cx = st.tile([128, H * T], F16)
        nc.sync.dma_start(cx[:], cx_in)
        cy = st.tile([128, H * T], F16)
        nc.sync.dma_start(cy[:], cy_in)
        i32 = st.tile([128, 128], F32)
        nc.sync.dma_start(i32[:], i32_in)
        i16t = st.tile([128, 128], F16)
        nc.sync.dma_start(i16t[:], i16_in)
        bmain = st.tile([128, 1], F32)
        nc.sync.dma_start(bmain[:], b_in)
        boff = st.tile([18, 1], F32)
        nc.sync.dma_start(boff[:], boff_in)

        scm = [st.tile([128, SCMW], F16, tag=f"scm{r}", name=f"scm{r}")
               for r in range(3)]
        for r in range(3):
            nc.vector.memset(scm[r][:], 0)

        # persistent per-image index + weight stores (filled in phase A)
        idxb_all = st.tile([128, BANDS * 576], I16, name="idxb_all")
        w36_all = st.tile([128, BANDS * 288], F16, name="w36_all")

        tok_src = bass.AP(xpair.tensor, 0, [[128, NPOS - 1], [1, 256]])

        def ap_of(tl, off, dims):
            b = tl[:]
            return bass.AP(b.tensor, b.offset + off, [b.ap[0]] + dims)

        def conv_band(b, scm_b):
            """main conv + channel-major store for band b reading scm_b."""
            for ch in range(2):          # two 512-px chunks (4 rows each)
                rb = 4 * ch              # starting row within band
                pc = psC.tile([128, 512], F32, tag="conv")
                n_mm = 45
                k = 0
                for s in range(9):
                    sy, sx = s // 3, s % 3
                    for kb in range(KB):
                        kdim = 128 if kb < 4 else 64
                        lhs = wm[0:kdim, (s * KB + kb) * 128:(s * KB + kb + 1) * 128]
                        rhs = ap_of(scm_b, kb * 10 * SLOT + (rb + sy) * SLOT + sx,
                                    [[SLOT, 4], [1, 128]])
                        rhs = bass.AP(rhs.tensor, rhs.offset,
                                      [[rhs.ap[0][0], kdim]] + rhs.ap[1:])
                        nc.tensor.matmul(
                            pc[:].rearrange("f (r x) -> f r x", r=4), lhs, rhs,
                            start=(k == 0), stop=(k == n_mm - 1))
                        k += 1
                outF = p_out.tile([128, 512], F32, tag="outF")
                nc.scalar.activation(outF[:], pc[:], ACTF.Identity,
                                     bias=bmain[:], scale=1.0)
                base = (b * ROWS_PER_BAND + 4 * ch) * PXROW
                dst = bass.AP(out_dram.tensor, base, [[H * WD, 128], [1, 512]])
                nc.sync.dma_start(dst, outF[:])

        # ================= phase A: indices + weights for all bands =========
        for b in range(BANDS):
            # ---------- offsets conv ----------
            offs_cm = []
            for ch in range(2):
                R = b * ROWS_PER_BAND + 4 * ch
                pa = psA.tile([18, 512], F32, tag="a")
                k = 0
                for ky in range(3):
                    rhs_d = ap_of(xdup, (R + ky) * PAD, [[PAD, 4], [1, 128]])
                    nc.tensor.matmul(
                        pa[:].rearrange("m (r x) -> m r x", r=4),
                        woffd[:, ky * 18:(ky + 1) * 18], rhs_d,
                        start=(k == 0), stop=False)
                    k += 1
                    rhs_s = bass.AP(
                        xdup[:].tensor, xdup[:].offset + (R + ky) * PAD + 2,
                        [[xdup[:].ap[0][0], 64], [PAD, 4], [1, 128]])
                    nc.tensor.matmul(
                        pa[:].rearrange("m (r x) -> m r x", r=4),
                        woffs[:, ky * 18:(ky + 1) * 18], rhs_s,
                        start=False, stop=(ky == 2))
                oc = p_offs.tile([18, 512], F32)
                nc.scalar.activation(oc[:], pa[:], ACTF.Identity,
                                     bias=boff[:], scale=1.0)
                offs_cm.append(oc)
            # ---------- offsets transpose to px-major ----------
            pt = psA.tile([128, 144], F32, tag="a")
            for r in range(ROWS_PER_BAND):
                lhs = offs_cm[r // 4][:, (r % 4) * 128:(r % 4 + 1) * 128]
                nc.tensor.matmul(pt[:, r * 18:(r + 1) * 18], lhs, i32[0:18, 0:18],
                                 start=True, stop=True)
            ob = p_offb.tile([128, 144], F32)
            nc.vector.tensor_copy(ob[:], pt[:])

            # ---------- bilinear weights + indices (px-major) ----------
            NW = ROWS_PER_BAND * T  # 72
            offx = ap_of(ob, 0, [[18, 8], [1, 9]])
            offy = ap_of(ob, 9, [[18, 8], [1, 9]])
            cxs = cx[:, b * NW:(b + 1) * NW]
            cys = cy[:, b * NW:(b + 1) * NW]

            def floor_block(off_ap, cs, hi_clip):
                l = p_math.tile([128, NW], F32, tag="l")
                nc.vector.tensor_tensor(l[:], off_ap, cs, ALU.add)
                nc.vector.tensor_scalar(l[:], l[:], 0.0, float(hi_clip),
                                        ALU.max, ALU.min)
                xi = p_math.tile([128, NW], I16, tag="xi")
                nc.vector.tensor_copy(xi[:], l[:])
                x0 = p_math.tile([128, NW], F32, tag="x0")
                nc.vector.tensor_copy(x0[:], xi[:])
                cg = p_math.tile([128, NW], F32, tag="cg")
                nc.vector.tensor_tensor(cg[:], x0[:], l[:], ALU.is_gt)
                nc.vector.tensor_tensor(x0[:], x0[:], cg[:], ALU.subtract)
                fx = p_math.tile([128, NW], F32, tag="fx")
                nc.vector.tensor_tensor(fx[:], l[:], x0[:], ALU.subtract)
                mx = p_math.tile([128, NW], F32, tag="mx")
                nc.vector.tensor_scalar(mx[:], x0[:], float(hi_clip - 1), None,
                                        ALU.is_le)
                wxa = p_math.tile([128, NW], F32, tag="wxa")
                nc.vector.tensor_scalar(wxa[:], fx[:], -1.0, 1.0, ALU.mult, ALU.add)
                nc.vector.tensor_tensor(wxa[:], wxa[:], mx[:], ALU.mult)
                return x0, fx, wxa

            x0, fx, wxa = floor_block(offx, cxs, 127)
            y0, fy, wya = floor_block(offy, cys, 127)

            # corner-weight store, interleaved (r, t, corner[a,b,c,d]) fp16
            wb_ = b * 288
            nc.vector.tensor_tensor(ap_of(w36_all, wb_ + 0, [[4, NW]]),
                                    wxa[:], wya[:], ALU.mult)
            nc.vector.tensor_tensor(ap_of(w36_all, wb_ + 1, [[4, NW]]),
                                    wxa[:], fy[:], ALU.mult)
            nc.vector.tensor_tensor(ap_of(w36_all, wb_ + 2, [[4, NW]]),
                                    fx[:], wya[:], ALU.mult)
            nc.vector.tensor_tensor(ap_of(w36_all, wb_ + 3, [[4, NW]]),
                                    fx[:], fy[:], ALU.mult)
            i0f = p_w.tile([128, NW], F32, tag="i0f")
            nc.vector.scalar_tensor_tensor(i0f[:], y0[:], 128.0, x0[:],
                                           op0=ALU.mult, op1=ALU.add)

            # ---------- index fold to wrapped gather layout ----------
            p1 = psA.tile([72, 128], F32, tag="a")
            nc.tensor.matmul(p1[:], i0f[:], i32[:], start=True, stop=True)
            t1 = p_fold.tile([72, 128], F32, tag="t1")
            nc.vector.tensor_copy(t1[:], p1[:])
            idx16 = p_fold.tile([16, ROWS_PER_BAND * T * 8], I16, tag="idx16")
            p2a = psA.tile([16, 288], F32, tag="a")
            p2b = psA.tile([16, 288], F32, tag="a")
            for a in range(8):
                p2 = p2a if a < 4 else p2b
                aa = a % 4
                nc.tensor.matmul(p2[:, aa * 72:(aa + 1) * 72],
                                 t1[:, 16 * a:16 * (a + 1)], i32[0:72, 0:72],
                                 start=True, stop=True)
                dst = ap_of(idx16, a, [[72, 8], [8, 9]])
                nc.vector.tensor_copy(dst, p2[:, aa * 72:(aa + 1) * 72])
            for g in range(8):
                nc.sync.dma_start(idxb_all[16 * g:16 * (g + 1),
                                           b * 576:(b + 1) * 576], idx16[:])
            if STAGE <= 1 and b == 0:
                w36f = p_w.tile([128, 288], F32, tag="w36f")
                nc.vector.tensor_copy(w36f[:], ap_of(w36_all, 0, [[1, 288]]))
                nc.sync.dma_start(dbg[:, 0:288], w36f[:])
                nc.sync.dma_start(dbg[:, 288:360], i0f[:])
                dbg16 = p_w.tile([128, 576], F32, tag="dbg16")
                nc.vector.tensor_copy(dbg16[:],
                                      ap_of(idxb_all, 0, [[1, 576]]))
                nc.sync.dma_start(dbg[:, 576:1152], dbg16[:])
        if STAGE <= 1:
            return

        # ================= phase B: gather / combine / conv =================
        gcall = [0]
        for b in range(BANDS):
            scm_b = scm[b % 3]
            for u in range(UNITS):
                gt = p_gt.tile([128, 18 * 256], F16)
                nn = IDX_PER_UNIT
                j0 = 0
                while nn > 0:
                    nidx = min(GCHUNK, nn)
                    gsl = gt[:, j0 * 256:j0 * 256 + (nidx // 128) * 256]
                    nc.gpsimd.dma_gather(
                        out_ap=gsl.rearrange("p (g e) -> p g e", g=nidx // 128),
                        in_ap=tok_src,
                        idxs_ap=idxb_all[:, b * 576 + u * 144 + j0 * 8:
                                         b * 576 + u * 144 + j0 * 8 + nidx // 16],
                        num_idxs=nidx,
                        num_idxs_reg=nidx,
                        elem_size=256,
                        elem_step=128,
                        single_packet=SINGLE_PACKET,
                        queue_num=gcall[0] % NQUEUES,
                    )
                    gcall[0] += 1
                    j0 += nidx // 128
                    nn -= nidx
                if STAGE <= 2:
                    if b == 0 and u == 0:
                        gdbg = p_dbg.tile([128, 4608], F32, tag="gdbg")
                        nc.vector.tensor_copy(gdbg[:], gt[:])
                        nc.sync.dma_start(dbg[:, 0:4608], gdbg[:])
                    continue
                # broadcast corner weights across the 64 channels (0-stride)
                w4 = p_w4.tile([128, 18 * 256], F16)
                nc.scalar.activation(
                    w4[:].rearrange("p (k c) -> p k c", c=64),
                    ap_of(w36_all, b * 288 + u * 72, [[1, 72], [0, 64]]),
                    ACTF.Copy)
                # combine: P = gt*w4, then fold 4 corners -> sampled fp16
                pp = p_cmb.tile([128, 18 * 256], F16, tag="pp")
                nc.vector.tensor_tensor(pp[:], gt[:], w4[:], ALU.mult)
                aa2 = p_cmb.tile([128, 18 * 128], F16, tag="aa2")
                nc.vector.tensor_tensor(
                    aa2[:], ap_of(pp, 0, [[256, 18], [1, 128]]),
                    ap_of(pp, 128, [[256, 18], [1, 128]]), ALU.add)
                spx = p_spx.tile([128, 2 * 576], F16)
                nc.vector.tensor_tensor(
                    spx[:], ap_of(aa2, 0, [[128, 18], [1, 64]]),
                    ap_of(aa2, 64, [[128, 18], [1, 64]]), ALU.add)
                # sampled transpose to channel-major
                for hi in range(2):
                    slot = 2 * u + hi + 1
                    ps1 = psS.tile([128, 512], F32, tag="s1")
                    ps2 = psB.tile([64, 128], F32, tag="b")
                    for kb in range(4):
                        nc.tensor.matmul(
                            ps1[:, kb * 128:(kb + 1) * 128],
                            spx[:, hi * 576 + kb * 128: hi * 576 + (kb + 1) * 128],
                            i16t[:], start=True, stop=True)
                    nc.tensor.matmul(ps2[:], spx[:, hi * 576 + 512:hi * 576 + 576],
                                     i16t[:], start=True, stop=True)
                    dst1 = ap_of(scm_b, slot * SLOT + 1, [[10 * SLOT, 4], [1, 128]])
                    nc.scalar.activation(dst1, ps1[:].rearrange(
                        "p (k x) -> p k x", k=4), ACTF.Copy)
                    dst2 = bass.AP(scm_b[:].tensor,
                                   scm_b[:].offset + 4 * 10 * SLOT + slot * SLOT + 1,
                                   [[scm_b[:].ap[0][0], 64], [1, 128]])
                    nc.scalar.activation(dst2, ps2[:], ACTF.Copy)
            if STAGE <= 2:
                continue
            if STAGE <= 3:
                if b == 0:
                    sdbg = p_dbg.tile([128, 4608], F32, tag="sdbg")
                    nc.vector.tensor_copy(sdbg[:], scm_b[:, 0:4608])
                    nc.sync.dma_start(dbg[:], sdbg[:])
                continue
            # halo: slot0 of this band from previous band's slot 8
            if b > 0:
                prev = scm[(b - 1) % 3]
                nc.vector.tensor_copy(
                    ap_of(scm_b, 0, [[10 * SLOT, KB], [1, SLOT]]),
                    ap_of(prev, 8 * SLOT, [[10 * SLOT, KB], [1, SLOT]]))
                nc.vector.tensor_copy(
                    ap_of(prev, 9 * SLOT, [[10 * SLOT, KB], [1, SLOT]]),
                    ap_of(scm_b, 1 * SLOT, [[10 * SLOT, KB], [1, SLOT]]))
                conv_band(b - 1, prev)
            if b == BANDS - 1:
                nc.vector.memset(
                    ap_of(scm_b, 9 * SLOT, [[10 * SLOT, KB], [1, SLOT]]), 0)
                conv_band(b, scm_b)


def _host_prep(x_img, W_off, b_off, W, b):
    """Build per-core input map. x_img: (128,128,64) fp32."""
    C_, T_ = C, T
    # row-paired fp16 token image: pos y*128+x -> [img[y,x,:], img[y+1,x,:]]
    xh = np.ascontiguousarray(x_img, np.float32).astype(np.float16)
    xpair = np.zeros((NPOS, 128), np.float16)
    xpair[:H * WD, :C_] = xh.reshape(H * WD, C_)
    xpair[:(H - 1) * WD, C_:] = xh[1:].reshape((H - 1) * WD, C_)
    xpair[(H - 1) * WD:H * WD, C_:] = xh[H - 1].reshape(WD, C_)

    # padded transposed image + dup(+1 col) for offset conv
    xT = np.zeros((C_, PAD, PAD), np.float16)
    xT[:, 1:129, 1:129] = np.transpose(x_img, (2, 0, 1)).astype(np.float16)
    xT = xT.reshape(C_, PAD * PAD)
    xdup = np.zeros((128, PAD * PAD), np.float16)
    xdup[:C_] = xT
    xdup[C_:, :PAD * PAD - 1] = xT[:, 1:]

    perm = list(range(0, 18, 2)) + list(range(1, 18, 2))
    woffd = np.zeros((128, 3 * 18), np.float16)
    woffs = np.zeros((64, 3 * 18), np.float16)
    for ky in range(3):
        woffd[:C_, ky * 18:(ky + 1) * 18] = W_off[ky, 0][:, perm].astype(np.float16)
        woffd[C_:, ky * 18:(ky + 1) * 18] = W_off[ky, 1][:, perm].astype(np.float16)
        woffs[:, ky * 18:(ky + 1) * 18] = W_off[ky, 2][:, perm].astype(np.float16)

    wm = np.zeros((128, 45 * 128), np.float16)
    for s in range(9):
        blk = W[s // 3, s % 3].astype(np.float16)        # [576, 128]
        for kb in range(KB):
            kd = 128 if kb < 4 else 64
            wm[:kd, (s * KB + kb) * 128:(s * KB + kb + 1) * 128] = \
                blk[kb * 128: kb * 128 + kd]

    lo = np.arange(128, dtype=np.float32)
    hi = np.arange(H, dtype=np.float32)
    t = np.arange(T_)
    kx = (t % 3 - 1).astype(np.float32)
    ky = (t // 3 - 1).astype(np.float32)
    cx = (lo[:, None, None] + kx[None, None, :] +
          np.zeros((1, H, 1), np.float32)).reshape(128, H * T_)
    cy = (np.zeros((128, 1, 1), np.float32) + hi[None, :, None] +
          ky[None, None, :]).reshape(128, H * T_)

    return dict(
        xpair=xpair.reshape(-1),
        xdup=xdup,
        woffd=woffd,
        woffs=woffs,
        wm=wm,
        cx=np.ascontiguousarray(cx).astype(np.float16),
        cy=np.ascontiguousarray(cy).astype(np.float16),
        i128f=np.eye(128, dtype=np.float32),
        i128h=np.eye(128, dtype=np.float16),
        b_main=np.asarray(b, np.float32).reshape(128, 1),
        b_off=np.asarray(b_off, np.float32)[
            list(range(0, 18, 2)) + list(range(1, 18, 2))].reshape(18, 1),
    )


def kernel(x, W_off, b_off, W, b, _trace=False):
    x = np.asarray(x, np.float32)
    nc = build_program()
    in_maps = [_host_prep(x[i], np.asarray(W_off, np.float32),
                          np.asarray(b_off, np.float32),
                          np.asarray(W, np.float32),
                          np.asarray(b, np.float32))
               for i in range(NCORES)]
    res = run_bass_kernel_spmd(nc, in_maps, list(range(NCORES)), trace=_trace)
    out = np.stack([res.results[i]["out"].reshape(F, H * WD).T.reshape(H, WD, F)
                    for i in range(NCORES)])
    if _trace:
        kernel.last_exec_time_ns = res.exec_time_ns
        kernel.last_results = res
    return out


kernel.last_exec_time_ns = None


# revision 27
# speedup vs baseline: 1.0499x; 1.0499x over previous
"""Deformable Conv2D Trainium2 kernel (8-core data-parallel over batch).

Per core (one image, H=W=128, C=64, F=128, 3x3 deformable conv):
  Phase A (all 16 bands up front, so gathers never wait):
    1. offset conv (PE, fp16, K-packed dual-tap matmuls)
    2. offsets transposed to pixel-major (PE identity matmuls)
    3. bilinear weights + gather indices (DVE, pixel-major)
    4. index fold to the DMA-gather wrapped layout (PE transposes + DVE)
       -> idxb_all [128, 16*576] i16, w36_all [128, 16*288] fp16
  Phase B (per band, Q7-gather-rate limited):
    5. dma_gather of 512B fp16 quad-corner tokens from the row-paired
       DRAM image (xpair[y*128+x] = img[y,x] ++ img[y+1,x]) - one
       descriptor per (pixel, tap)
    6. bilinear combine: corner weights broadcast across channels
       (0-stride ACT copy), one wide DVE multiply + 2 strided adds
    7. sampled transposed to channel-major via PE identity matmuls into a
       halo'd per-band buffer
    8. main conv: 45 accumulating PE matmuls per 512-px chunk (fp16)
    9. output stored channel-major [F, H*W]; host transposes to NHWC

Self-contained: hardcodes shapes for the nn_DeformableConv2D problem.
"""
import os
import numpy as np

import concourse.bass as bass
import concourse.bacc as bacc
import concourse.tile as tile
from concourse import mybir
from concourse.bass_utils import run_bass_kernel_spmd

F32, F16, I16 = mybir.dt.float32, mybir.dt.float16, mybir.dt.int16
ALU = mybir.AluOpType
ACTF = mybir.ActivationFunctionType

H = WD = 128
C = 64
F = 128
T = 9            # deformable taps
NCORES = 8
ROWS_PER_BAND = 8
BANDS = H // ROWS_PER_BAND          # 16
UNITS = 4                            # 2-row units per band
PXROW = WD                            # 128 px per image row
PAD = 130                            # padded row length for shifted reads
NPOS = 16512                         # xpair positions (129 rows x 128)
KB = 5                               # K blocks of main conv (576 -> 640)
SLOT = PAD                           # 130 cols per row slot in scm
SCMW = KB * 10 * SLOT                # 6500 cols per band buffer
IDX_PER_UNIT = 2 * T * PXROW         # 2304 tokens per 2-row unit

_CACHE = {}


STAGE = int(os.environ.get("KSTAGE", "4"))
GCHUNK = int(os.environ.get("KGCHUNK", "1024"))
SINGLE_PACKET = os.environ.get("KSP", "1") == "1"
NQUEUES = int(os.environ.get("KNQ", "4"))


def build_program():
    if "nc" in _CACHE:
        return _CACHE["nc"]
    nc = bacc.Bacc("TRN2", target_bir_lowering=False, debug=False,
                   num_swdge_queues=NQUEUES)

    # ---- DRAM I/O ----
    xpair = nc.dram_tensor("xpair", [NPOS * 128], F16, kind="ExternalInput").ap()
    xdup = nc.dram_tensor("xdup", [128, PAD * PAD], F16, kind="ExternalInput").ap()
    woffd_in = nc.dram_tensor("woffd", [128, 3 * 18], F16, kind="ExternalInput").ap()
    woffs_in = nc.dram_tensor("woffs", [64, 3 * 18], F16, kind="ExternalInput").ap()
    wm_in = nc.dram_tensor("wm", [128, 45 * 128], F16, kind="ExternalInput").ap()
    cx_in = nc.dram_tensor("cx", [128, H * T], F16, kind="ExternalInput").ap()
    cy_in = nc.dram_tensor("cy", [128, H * T], F16, kind="ExternalInput").ap()
    i32_in = nc.dram_tensor("i128f", [128, 128], F32, kind="ExternalInput").ap()
    i16_in = nc.dram_tensor("i128h", [128, 128], F16, kind="ExternalInput").ap()
    b_in = nc.dram_tensor("b_main", [128, 1], F32, kind="ExternalInput").ap()
    boff_in = nc.dram_tensor("b_off", [18, 1], F32, kind="ExternalInput").ap()
    out_dram = nc.dram_tensor("out", [F, H * WD], F32, kind="ExternalOutput").ap()
    dbg = nc.dram_tensor("dbg", [128, 4608], F32, kind="ExternalOutput").ap()

    with tile.TileContext(nc) as tc:
        _emit(nc, tc, xpair, xdup, woffd_in, woffs_in, wm_in, cx_in, cy_in,
              i32_in, i16_in, b_in, boff_in, out_dram, dbg)

    nc.compile()
    _CACHE["nc"] = nc
    return nc


def _emit(nc, tc, xpair, xdup_in, woffd_in, woffs_in, wm_in, cx_in, cy_in,
          i32_in, i16_in, b_in, boff_in, out_dram, dbg):
    from contextlib import ExitStack
    with ExitStack() as ctx:
        ec = ctx.enter_context
        st = ec(tc.tile_pool(name="static", bufs=1))
        p_offs = ec(tc.tile_pool(name="offs", bufs=2))
        p_offb = ec(tc.tile_pool(name="offb", bufs=1))
        p_math = ec(tc.tile_pool(name="math", bufs=2))
        p_w = ec(tc.tile_pool(name="wts", bufs=2))
        p_fold = ec(tc.tile_pool(name="fold", bufs=1))
        p_gt = ec(tc.tile_pool(name="gt", bufs=3))
        p_w4 = ec(tc.tile_pool(name="w4", bufs=2))
        p_cmb = ec(tc.tile_pool(name="cmb", bufs=2))
        p_spx = ec(tc.tile_pool(name="spx", bufs=2))
        p_out = ec(tc.tile_pool(name="outp", bufs=2))
        p_dbg = ec(tc.tile_pool(name="dbgp", bufs=1)) if STAGE <= 3 else None
        psA = ec(tc.tile_pool(name="psA", bufs=2, space="PSUM"))
        psB = ec(tc.tile_pool(name="psB", bufs=2, space="PSUM"))
        psS = ec(tc.tile_pool(name="psS", bufs=2, space="PSUM"))
        psC = ec(tc.tile_pool(name="psC", bufs=2, space="PSUM"))

        # ---- static loads (HWDGE; keep Q7 free for gathers) ----
        xdup = st.tile([128, PAD * PAD], F16)
        HEADC = 12 * PAD
        nc.sync.dma_start(xdup[:, 0:HEADC],
                          bass.AP(xdup_in.tensor, 0, [[PAD * PAD, 128], [1, HEADC]]))
        nc.sync.dma_start(xdup[:, HEADC:],
                          bass.AP(xdup_in.tensor, HEADC,
                                  [[PAD * PAD, 128], [1, PAD * PAD - HEADC]]))
        woffd = st.tile([128, 54], F16)
        nc.sync.dma_start(woffd[:], woffd_in)
        woffs = st.tile([64, 54], F16)
        nc.sync.dma_start(woffs[:], woffs_in)
        wm = st.tile([128, 45 * 128], F16)
        nc.sync.dma_start(wm[:], wm_in)
        cx = st.tile([128, H * T], F16)
        nc.sync.dma_start(cx[:], cx_in)
        cy = st.tile([128, H * T], F16)
        nc.sync.dma_start(cy[:], cy_in)
        i32 = st.tile([128, 128], F32)
        nc.sync.dma_start(i32[:], i32_in)
        i16t = st.tile([128, 128], F16)
        nc.sync.dma_start(i16t[:], i16_in)
        bmain = st.tile([128, 1], F32)
        nc.sync.dma_start(bmain[:], b_in)
        boff = st.tile([18, 1], F32)
        nc.sync.dma_start(boff[:], boff_in)

        scm = [st.tile([128, SCMW], F16, tag=f"scm{r}", name=f"scm{r}")
               for r in range(3)]
        for r in range(3):
            nc.vector.memset(scm[r][:], 0)

        # persistent per-image index + weight stores (filled in phase A)
        idxb_all = st.tile([128, BANDS * 576], I16, name="idxb_all")
        w36_all = st.tile([128, BANDS * 288], F16, name="w36_all")

        tok_src = bass.AP(xpair.tensor, 0, [[128, NPOS - 1], [1, 256]])

        def ap_of(tl, off, dims):
            b = tl[:]
            return bass.AP(b.tensor, b.offset + off, [b.ap[0]] + dims)

        def conv_band(b, scm_b):
            """main conv + channel-major store for band b reading scm_b."""
            for ch in range(2):          # two 512-px chunks (4 rows each)
                rb = 4 * ch              # starting row within band
                pc = psC.tile([128, 512], F32, tag="conv")
                n_mm = 45
                k = 0
                for s in range(9):
                    sy, sx = s // 3, s % 3
                    for kb in range(KB):
                        lhs = wm[:, (s * KB + kb) * 128:(s * KB + kb + 1) * 128]
                        rhs = ap_of(scm_b, kb * 10 * SLOT + (rb + sy) * SLOT + sx,
                                    [[SLOT, 4], [1, 128]])
                        nc.tensor.matmul(
                            pc[:].rearrange("f (r x) -> f r x", r=4), lhs, rhs,
                            start=(k == 0), stop=(k == n_mm - 1))
                        k += 1
                outF = p_out.tile([128, 512], F32, tag="outF")
                nc.scalar.activation(outF[:], pc[:], ACTF.Identity,
                                     bias=bmain[:], scale=1.0)
                base = (b * ROWS_PER_BAND + 4 * ch) * PXROW
                dst = bass.AP(out_dram.tensor, base, [[H * WD, 128], [1, 512]])
                nc.sync.dma_start(dst, outF[:])

        # ================= phase A: indices + weights for all bands =========
        for b in range(BANDS):
            # ---------- offsets conv ----------
            offs_cm = []
            for ch in range(2):
                R = b * ROWS_PER_BAND + 4 * ch
                pa = psA.tile([18, 512], F32, tag="a")
                k = 0
                for ky in range(3):
                    rhs_d = ap_of(xdup, (R + ky) * PAD, [[PAD, 4], [1, 128]])
                    nc.tensor.matmul(
                        pa[:].rearrange("m (r x) -> m r x", r=4),
                        woffd[:, ky * 18:(ky + 1) * 18], rhs_d,
                        start=(k == 0), stop=False)
                    k += 1
                    rhs_s = bass.AP(
                        xdup[:].tensor, xdup[:].offset + (R + ky) * PAD + 2,
                        [[xdup[:].ap[0][0], 64], [PAD, 4], [1, 128]])
                    nc.tensor.matmul(
                        pa[:].rearrange("m (r x) -> m r x", r=4),
                        woffs[:, ky * 18:(ky + 1) * 18], rhs_s,
                        start=False, stop=(ky == 2))
                oc = p_offs.tile([18, 512], F32)
                nc.scalar.activation(oc[:], pa[:], ACTF.Identity,
                                     bias=boff[:], scale=1.0)
                offs_cm.append(oc)
            # ---------- offsets transpose to px-major ----------
            pt = psA.tile([128, 144], F32, tag="a")
            for r in range(ROWS_PER_BAND):
                lhs = offs_cm[r // 4][:, (r % 4) * 128:(r % 4 + 1) * 128]
                nc.tensor.matmul(pt[:, r * 18:(r + 1) * 18], lhs, i32[0:18, 0:18],
                                 start=True, stop=True)
            ob = p_offb.tile([128, 144], F32)
            nc.vector.tensor_copy(ob[:], pt[:])

            # ---------- bilinear weights + indices (px-major) ----------
            NW = ROWS_PER_BAND * T  # 72
            offx = ap_of(ob, 0, [[18, 8], [1, 9]])
            offy = ap_of(ob, 9, [[18, 8], [1, 9]])
            cxs = cx[:, b * NW:(b + 1) * NW]
            cys = cy[:, b * NW:(b + 1) * NW]

            def floor_block(off_ap, cs, hi_clip):
                l = p_math.tile([128, NW], F32, tag="l")
                nc.vector.tensor_tensor(l[:], off_ap, cs, ALU.add)
                nc.vector.tensor_scalar(l[:], l[:], 0.0, float(hi_clip),
                                        ALU.max, ALU.min)
                xi = p_math.tile([128, NW], I16, tag="xi")
                nc.vector.tensor_copy(xi[:], l[:])
                x0 = p_math.tile([128, NW], F32, tag="x0")
                nc.vector.tensor_copy(x0[:], xi[:])
                cg = p_math.tile([128, NW], F32, tag="cg")
                nc.vector.tensor_tensor(cg[:], x0[:], l[:], ALU.is_gt)
                nc.vector.tensor_tensor(x0[:], x0[:], cg[:], ALU.subtract)
                fx = p_math.tile([128, NW], F32, tag="fx")
                nc.vector.tensor_tensor(fx[:], l[:], x0[:], ALU.subtract)
                mx = p_math.tile([128, NW], F32, tag="mx")
                nc.vector.tensor_scalar(mx[:], x0[:], float(hi_clip - 1), None,
                                        ALU.is_le)
                wxa = p_math.tile([128, NW], F32, tag="wxa")
                nc.vector.tensor_scalar(wxa[:], fx[:], -1.0, 1.0, ALU.mult, ALU.add)
                nc.vector.tensor_tensor(wxa[:], wxa[:], mx[:], ALU.mult)
                return x0, fx, wxa

            x0, fx, wxa = floor_block(offx, cxs, 127)
            y0, fy, wya = floor_block(offy, cys, 127)

            # corner-weight store, interleaved (r, t, corner[a,b,c,d]) fp16
            wb_ = b * 288
            nc.vector.tensor_tensor(ap_of(w36_all, wb_ + 0, [[4, NW]]),
                                    wxa[:], wya[:], ALU.mult)
            nc.vector.tensor_tensor(ap_of(w36_all, wb_ + 1, [[4, NW]]),
                                    wxa[:], fy[:], ALU.mult)
            nc.vector.tensor_tensor(ap_of(w36_all, wb_ + 2, [[4, NW]]),
                                    fx[:], wya[:], ALU.mult)
            nc.vector.tensor_tensor(ap_of(w36_all, wb_ + 3, [[4, NW]]),
                                    fx[:], fy[:], ALU.mult)
            i0f = p_w.tile([128, NW], F32, tag="i0f")
            nc.vector.scalar_tensor_tensor(i0f[:], y0[:], 128.0, x0[:],
                                           op0=ALU.mult, op1=ALU.add)

            # ---------- index fold to wrapped gather layout ----------
            p1 = psA.tile([72, 128], F32, tag="a")
            nc.tensor.matmul(p1[:], i0f[:], i32[:], start=True, stop=True)
            t1 = p_fold.tile([72, 128], F32, tag="t1")
            nc.vector.tensor_copy(t1[:], p1[:])
            idx16 = p_fold.tile([16, ROWS_PER_BAND * T * 8], I16, tag="idx16")
            p2a = psA.tile([16, 288], F32, tag="a")
            p2b = psA.tile([16, 288], F32, tag="a")
            for a in range(8):
                p2 = p2a if a < 4 else p2b
                aa = a % 4
                nc.tensor.matmul(p2[:, aa * 72:(aa + 1) * 72],
                                 t1[:, 16 * a:16 * (a + 1)], i32[0:72, 0:72],
                                 start=True, stop=True)
                dst = ap_of(idx16, a, [[72, 8], [8, 9]])
                nc.vector.tensor_copy(dst, p2[:, aa * 72:(aa + 1) * 72])
            for g in range(8):
                nc.sync.dma_start(idxb_all[16 * g:16 * (g + 1),
                                           b * 576:(b + 1) * 576], idx16[:])
            if STAGE <= 1 and b == 0:
                w36f = p_w.tile([128, 288], F32, tag="w36f")
                nc.vector.tensor_copy(w36f[:], ap_of(w36_all, 0, [[1, 288]]))
                nc.sync.dma_start(dbg[:, 0:288], w36f[:])
                nc.sync.dma_start(dbg[:, 288:360], i0f[:])
                dbg16 = p_w.tile([128, 576], F32, tag="dbg16")
                nc.vector.tensor_copy(dbg16[:],
                                      ap_of(idxb_all, 0, [[1, 576]]))
                nc.sync.dma_start(dbg[:, 576:1152], dbg16[:])
        if STAGE <= 1:
            return

        # ================= phase B: gather / combine / conv =================
        gcall = [0]
        for b in range(BANDS):
            scm_b = scm[b % 3]
            for u in range(UNITS):
                gt = p_gt.tile([128, 18 * 256], F16)
                nn = IDX_PER_UNIT
                j0 = 0
                while nn > 0:
                    nidx = min(GCHUNK, nn)
                    gsl = gt[:, j0 * 256:j0 * 256 + (nidx // 128) * 256]
                    nc.gpsimd.dma_gather(
                        out_ap=gsl.rearrange("p (g e) -> p g e", g=nidx // 128),
                        in_ap=tok_src,
                        idxs_ap=idxb_all[:, b * 576 + u * 144 + j0 * 8:
                                         b * 576 + u * 144 + j0 * 8 + nidx // 16],
                        num_idxs=nidx,
                        num_idxs_reg=nidx,
                        elem_size=256,
                        elem_step=128,
                        single_packet=SINGLE_PACKET,
                        queue_num=gcall[0] % NQUEUES,
                    )
                    gcall[0] += 1
                    j0 += nidx // 128
                    nn -= nidx
                if STAGE <= 2:
                    if b == 0 and u == 0:
                        gdbg = p_dbg.tile([128, 4608], F32, tag="gdbg")
                        nc.vector.tensor_copy(gdbg[:], gt[:])
                        nc.sync.dma_start(dbg[:, 0:4608], gdbg[:])
                    continue
                # broadcast corner weights across the 64 channels (0-stride)
                w4 = p_w4.tile([128, 18 * 256], F16)
                nc.scalar.activation(
                    w4[:].rearrange("p (k c) -> p k c", c=64),
                    ap_of(w36_all, b * 288 + u * 72, [[1, 72], [0, 64]]),
                    ACTF.Copy)
                # combine: P = gt*w4, then fold 4 corners -> sampled fp16
                pp = p_cmb.tile([128, 18 * 256], F16, tag="pp")
                nc.vector.tensor_tensor(pp[:], gt[:], w4[:], ALU.mult)
                aa2 = p_cmb.tile([128, 18 * 128], F16, tag="aa2")
                nc.vector.tensor_tensor(
                    aa2[:], ap_of(pp, 0, [[256, 18], [1, 128]]),
                    ap_of(pp, 128, [[256, 18], [1, 128]]), ALU.add)
                spx = p_spx.tile([128, 2 * 576], F16)
                nc.vector.tensor_tensor(
                    spx[:], ap_of(aa2, 0, [[128, 18], [1, 64]]),
                    ap_of(aa2, 64, [[128, 18], [1, 64]]), ALU.add)
                # sampled transpose to channel-major
                for hi in range(2):
                    slot = 2 * u + hi + 1
                    ps1 = psS.tile([128, 512], F32, tag="s1")
                    ps2 = psB.tile([64, 128], F32, tag="b")
                    for kb in range(4):
                        nc.tensor.matmul(
                            ps1[:, kb * 128:(kb + 1) * 128],
                            spx[:, hi * 576 + kb * 128: hi * 576 + (kb + 1) * 128],
                            i16t[:], start=True, stop=True)
                    nc.tensor.matmul(ps2[:], spx[:, hi * 576 + 512:hi * 576 + 576],
                                     i16t[:], start=True, stop=True)
                    dst1 = ap_of(scm_b, slot * SLOT + 1, [[10 * SLOT, 4], [1, 128]])
                    nc.scalar.activation(dst1, ps1[:].rearrange(
                        "p (k x) -> p k x", k=4), ACTF.Copy)
                    dst2 = bass.AP(scm_b[:].tensor,
                                   scm_b[:].offset + 4 * 10 * SLOT + slot * SLOT + 1,
                                   [[scm_b[:].ap[0][0], 64], [1, 128]])
                    nc.scalar.activation(dst2, ps2[:], ACTF.Copy)
            if STAGE <= 2:
                continue
            if STAGE <= 3:
                if b == 0:
                    sdbg = p_dbg.tile([128, 4608], F32, tag="sdbg")
                    nc.vector.tensor_copy(sdbg[:], scm_b[:, 0:4608])
                    nc.sync.dma_start(dbg[:], sdbg[:])
                continue
            # halo: slot0 of this band from previous band's slot 8
            if b > 0:
                prev = scm[(b - 1) % 3]
                nc.vector.tensor_copy(
                    ap_of(scm_b, 0, [[10 * SLOT, KB], [1, SLOT]]),
                    ap_of(prev, 8 * SLOT, [[10 * SLOT, KB], [1, SLOT]]))
                nc.vector.tensor_copy(
                    ap_of(prev, 9 * SLOT, [[10 * SLOT, KB], [1, SLOT]]),
                    ap_of(scm_b, 1 * SLOT, [[10 * SLOT, KB], [1, SLOT]]))
                conv_band(b - 1, prev)
            if b == BANDS - 1:
                nc.vector.memset(
                    ap_of(scm_b, 9 * SLOT, [[10 * SLOT, KB], [1, SLOT]]), 0)
                conv_band(b, scm_b)


def _host_prep(x_img, W_off, b_off, W, b):
    """Build per-core input map. x_img: (128,128,64) fp32."""
    C_, T_ = C, T
    # row-paired fp16 token image: pos y*128+x -> [img[y,x,:], img[y+1,x,:]]
    xh = np.ascontiguousarray(x_img, np.float32).astype(np.float16)
    xpair = np.zeros((NPOS, 128), np.float16)
    xpair[:H * WD, :C_] = xh.reshape(H * WD, C_)
    xpair[:(H - 1) * WD, C_:] = xh[1:].reshape((H - 1) * WD, C_)
    xpair[(H - 1) * WD:H * WD, C_:] = xh[H - 1].reshape(WD, C_)

    # padded transposed image + dup(+1 col) for offset conv
    xT = np.zeros((C_, PAD, PAD), np.float16)
    xT[:, 1:129, 1:129] = np.transpose(x_img, (2, 0, 1)).astype(np.float16)
    xT = xT.reshape(C_, PAD * PAD)
    xdup = np.zeros((128, PAD * PAD), np.float16)
    xdup[:C_] = xT
    xdup[C_:, :PAD * PAD - 1] = xT[:, 1:]

    perm = list(range(0, 18, 2)) + list(range(1, 18, 2))
    woffd = np.zeros((128, 3 * 18), np.float16)
    woffs = np.zeros((64, 3 * 18), np.float16)
    for ky in range(3):
        woffd[:C_, ky * 18:(ky + 1) * 18] = W_off[ky, 0][:, perm].astype(np.float16)
        woffd[C_:, ky * 18:(ky + 1) * 18] = W_off[ky, 1][:, perm].astype(np.float16)
        woffs[:, ky * 18:(ky + 1) * 18] = W_off[ky, 2][:, perm].astype(np.float16)

    wm = np.zeros((128, 45 * 128), np.float16)
    for s in range(9):
        blk = W[s // 3, s % 3].astype(np.float16)        # [576, 128]
        for kb in range(KB):
            kd = 128 if kb < 4 else 64
            wm[:kd, (s * KB + kb) * 128:(s * KB + kb + 1) * 128] = \
                blk[kb * 128: kb * 128 + kd]

    lo = np.arange(128, dtype=np.float32)
    hi = np.arange(H, dtype=np.float32)
    t = np.arange(T_)
    kx = (t % 3 - 1).astype(np.float32)
    ky = (t // 3 - 1).astype(np.float32)
    cx = (lo[:, None, None] + kx[None, None, :] +
          np.zeros((1, H, 1), np.float32)).reshape(128, H * T_)
    cy = (np.zeros((128, 1, 1), np.float32) + hi[None, :, None] +
          ky[None, None, :]).reshape(128, H * T_)

    return dict(
        xpair=xpair.reshape(-1),
        xdup=xdup,
        woffd=woffd,
        woffs=woffs,
        wm=wm,
        cx=np.ascontiguousarray(cx).astype(np.float16),
        cy=np.ascontiguousarray(cy).astype(np.float16),
        i128f=np.eye(128, dtype=np.float32),
        i128h=np.eye(128, dtype=np.float16),
        b_main=np.asarray(b, np.float32).reshape(128, 1),
        b_off=np.asarray(b_off, np.float32)[
            list(range(0, 18, 2)) + list(range(1, 18, 2))].reshape(18, 1),
    )


def kernel(x, W_off, b_off, W, b, _trace=False):
    x = np.asarray(x, np.float32)
    nc = build_program()
    in_maps = [_host_prep(x[i], np.asarray(W_off, np.float32),
                          np.asarray(b_off, np.float32),
                          np.asarray(W, np.float32),
                          np.asarray(b, np.float32))
               for i in range(NCORES)]
    res = run_bass_kernel_spmd(nc, in_maps, list(range(NCORES)), trace=_trace)
    out = np.stack([res.results[i]["out"].reshape(F, H * WD).T.reshape(H, WD, F)
                    for i in range(NCORES)])
    if _trace:
        kernel.last_exec_time_ns = res.exec_time_ns
        kernel.last_results = res
    return out


kernel.last_exec_time_ns = None
